# revision 46
# baseline (speedup 1.0000x reference)
"""FLUX-style joint attention block (SVDQuant linears) on 8 TRN2 NeuronCores.

Strategy (tensor parallel over heads, per the sharding hint):
  - 24 heads -> 3 heads per core. Each core computes its QKV column shard
    from the full (replicated) hidden states, runs attention for its heads,
    then computes its 384 output-feature rows of the output projections in a
    transposed formulation. Head outputs are exchanged with a per-chunk
    AllGather of attn^T (cheaper than all-reducing the full output, and
    overlapped with attention compute of the next chunk).
  - All GEMMs in bf16 with fp32 PSUM accumulation. RMS-norm + RoPE applied
    in fp32 on the transposed [head_dim, seq] layout: partition-axis sums
    via ones-matmul, RoPE pair swap via a permutation matmul, per-column
    scale broadcast via gpsimd.partition_broadcast.
  - softmax without max-subtraction (scores are O(5) here); denominators
    from bf16 tree-sum + ones-matmul; normalization folded into the PSUM
    eviction of P@V.

Self-contained: hardcodes shapes from the problem spec.
"""

import os
import sys
from contextlib import ExitStack

import numpy as np
import ml_dtypes

for _p in ("/opt/trn_rl_repo",):
    if os.path.isdir(_p) and _p not in sys.path:
        sys.path.append(_p)

import concourse.bass as bass  # noqa: E402
import concourse.mybir as mybir  # noqa: E402
import concourse.tile as tile  # noqa: E402
from concourse.tile import add_dep_helper  # noqa: E402
from concourse import bacc  # noqa: E402
from concourse import tile_utils  # noqa: E402
from concourse.bass_utils import run_bass_kernel_spmd  # noqa: E402

# Use the full usable SBUF (224 KiB phys / 208 usable per partition).
tile_utils.max_sbuf_usage = 208 * 1024

N_CORES = 8
S_TXT, S_IMG = 512, 2048
S = S_TXT + S_IMG                      # 2560
QD = 3072                              # model dim
HEADS, DH, RANK = 24, 128, 32
H_PC = HEADS // N_CORES                # 3 heads per core
IPC = H_PC * DH                        # 384 inner dims per core
KT = QD // 128                         # 24 contraction tiles
NT = S // 512                          # 5 seq tiles of 512 (tile 0 = txt)
HALVES = [(0, 3), (3, 2)]              # (first n-tile, n n-tiles): 1536 + 1024
SCALE = 1.0 / float(np.sqrt(np.float32(DH)))
EPS = 1e-5

BF16 = mybir.dt.bfloat16
F32 = mybir.dt.float32
AF = mybir.ActivationFunctionType

_CACHE = {}


# ---------------------------------------------------------------- host packing

def _bf(a):
    return np.ascontiguousarray(a.astype(ml_dtypes.bfloat16))


def _f32(a):
    return np.ascontiguousarray(a.astype(np.float32))


def _pack_st(a):
    """[3072, M] -> [128, KT*M] with block kt at columns [kt*M, (kt+1)*M)."""
    m = a.shape[1]
    return a.reshape(KT, 128, m).transpose(1, 0, 2).reshape(128, KT * m)


def _prep_inputs(inp):
    """Build the 8 per-core input maps (all np arrays, bf16/f32)."""
    x = np.concatenate([inp["encoder_hidden_states"][0],
                        inp["hidden_states"][0]], axis=0).astype(np.float32)

    # xT packed n-tile-major: block j is [128, KT*512], kt-blocks inside.
    xt_blocks = []
    for j in range(NT):
        xs = x[j * 512:(j + 1) * 512]             # [512, 3072]
        xt_blocks.append(xs.T.reshape(KT, 128, 512)
                         .transpose(1, 0, 2).reshape(128, KT * 512))
    xt_pack = _bf(np.concatenate(xt_blocks, axis=1))

    rope = np.asarray(inp["image_rotary_emb"], np.float32)   # [S, 64, 2]
    cos = np.repeat(rope[:, :, 0], 2, axis=1)                # [S, 128]
    sinE = np.empty((S, DH), np.float32)
    sinE[:, 0::2] = -rope[:, :, 1]
    sinE[:, 1::2] = rope[:, :, 1]
    txt_rows = (np.arange(S) < S_TXT)[:, None]
    gq = np.where(txt_rows, inp["g_aq"][None, :], inp["g_q"][None, :])
    gk = np.where(txt_rows, inp["g_ak"][None, :], inp["g_k"][None, :])
    swap = lambda a: a[:, np.arange(DH) ^ 1]
    cs = {
        "cos_q": (cos * gq).T, "sin_q": (sinE * swap(gq)).T,
        "cos_k": (cos * gk).T, "sin_k": (sinE * swap(gk)).T,
    }  # each [128, S] fp32

    perm = np.zeros((128, 128), np.float32)
    perm[np.arange(128), np.arange(128) ^ 1] = 1.0
    ones = np.ones((128, 1), np.float32)

    # fold the SVD low-rank branch into the weights: w_eff = w + pd @ pu
    w_qkv = inp["w_qkv"] + inp["pd_qkv"] @ inp["pu_qkv"]
    w_aqkv = inp["w_add_qkv"] + inp["pd_add_qkv"] @ inp["pu_add_qkv"]
    w_out = inp["w_out"] + inp["pd_out"] @ inp["pu_out"]
    w_aout = inp["w_add_out"] + inp["pd_add_out"] @ inp["pu_add_out"]

    in_maps = []
    for c in range(N_CORES):
        heads = [3 * c + j for j in range(H_PC)]
        qk_cols = np.concatenate(
            [np.arange(h * 128, (h + 1) * 128) for h in heads]
            + [np.arange(QD + h * 128, QD + (h + 1) * 128) for h in heads])
        v_cols = np.concatenate(
            [np.arange(2 * QD + h * 128, 2 * QD + (h + 1) * 128)
             for h in heads])
        qd_cols = np.arange(c * IPC, (c + 1) * IPC)

        def pack_mtiles(w, cols, mt):
            sel = w[:, cols]
            return np.concatenate(
                [_pack_st(sel[:, m * 128:(m + 1) * 128]) for m in range(mt)],
                axis=1)

        m = {
            "xt": xt_pack,
            "wqk_img": _bf(pack_mtiles(w_qkv, qk_cols, 6)),
            "wqk_txt": _bf(pack_mtiles(w_aqkv, qk_cols, 6)),
            "wv_img": _bf(_pack_st(w_qkv[:, v_cols])),
            "wv_txt": _bf(_pack_st(w_aqkv[:, v_cols])),
            "wo_img": _bf(pack_mtiles(w_out, qd_cols, 3)),
            "wo_txt": _bf(pack_mtiles(w_aout, qd_cols, 3)),
            "bias_img": _f32(inp["b_out"][qd_cols].reshape(3, 128).T),
            "bias_txt": _f32(inp["b_add_out"][qd_cols].reshape(3, 128).T),
            "cos_q": _f32(cs["cos_q"]), "sin_q": _f32(cs["sin_q"]),
            "cos_k": _f32(cs["cos_k"]), "sin_k": _f32(cs["sin_k"]),
            "permT": _bf(perm), "ones128": _bf(ones),
        }
        in_maps.append(m)
    return in_maps


# ---------------------------------------------------------------- device build

def _declare(nc):
    d = {}
    def inp(name, shape, dt=BF16):
        d[name] = nc.dram_tensor(name, list(shape), dt, kind="ExternalInput")
    inp("xt", [128, NT * KT * 512])
    inp("wqk_img", [128, 6 * KT * 128]); inp("wqk_txt", [128, 6 * KT * 128])
    inp("wv_img", [128, KT * IPC]); inp("wv_txt", [128, KT * IPC])
    inp("wo_img", [128, 3 * KT * 128]); inp("wo_txt", [128, 3 * KT * 128])
    inp("bias_img", [128, 3], F32); inp("bias_txt", [128, 3], F32)
    for n in ("cos_q", "sin_q", "cos_k", "sin_k"):
        inp(n, [128, S], F32)
    inp("permT", [128, 128]); inp("ones128", [128, 1])
    d["yt_out"] = nc.dram_tensor("yt_out", [IPC, S], F32,
                                 kind="ExternalOutput")
    return d


def _build():
    nc = bacc.Bacc("TRN2", target_bir_lowering=False, debug=False,
                   num_devices=N_CORES)
    d = _declare(nc)

    with tile.TileContext(nc) as tc, ExitStack() as ctx:
        persist = ctx.enter_context(tc.tile_pool(name="persist", bufs=1))
        # persistent tensors
        qk_sb = persist.tile([128, 6 * S], BF16, tag="qk")   # [T][d, seq]
        v_sb = persist.tile([128, NT * 4 * IPC], BF16, tag="v")  # 20 seq tiles
        ones_sb = persist.tile([128, 1], BF16, tag="ones")
        perm_sb = persist.tile([128, 128], BF16, tag="perm")
        eps_sb = persist.tile([1, 1], F32, tag="eps")
        nc.sync.dma_start(ones_sb[:], d["ones128"][:])
        nc.sync.dma_start(perm_sb[:], d["permT"][:])
        nc.vector.memset(eps_sb[:], EPS)

        # ---------------- phase 1: QKV + rms/rope, n-tile-outer ------------
        with (
            tc.tile_pool(name="qkvtmp", bufs=1) as qp,
            tc.tile_pool(name="qkvpsum", bufs=1, space="PSUM") as pp,
        ):
            wv = {}

            for jg in ((0,), (1, 2), (3, 4)):
                dom = "txt" if jg[0] == 0 else "img"
                xts = {}
                first_group = (jg == (0,))
                for j in jg:
                    xts[j] = qp.tile([128, KT * 512], BF16, tag="xt",
                                     bufs=3, name=f"xt{j}")
                    # split so the first k-tiles land early and matmuls start
                    cut = 4 * 512
                    nc.sync.dma_start(
                        xts[j][:, :cut],
                        d["xt"][:, j * KT * 512:j * KT * 512 + cut])
                    nc.sync.dma_start(
                        xts[j][:, cut:],
                        d["xt"][:, j * KT * 512 + cut:(j + 1) * KT * 512])

                # Q,K transposed GEMM; one LDWEIGHTS serves the j-pair.
                # rms ln/exp batched over T-groups of 3 to avoid ACT table
                # set thrashing between Ln and Exp.
                for Tg in ((0, 1, 2), (3, 4, 5)):
                    rms_ins, t12s = {}, {}
                    for j in jg:
                        rms_ins[j] = qp.tile([1, 3 * 512], F32, tag="rms_in",
                                             bufs=2, name=f"rms_in{j}")
                    for T in Tg:
                        qk = "q" if T < 3 else "k"
                        wqT = qp.tile([128, KT * 128], BF16, tag="wq",
                                      bufs=2, name="wqT")
                        nc.sync.dma_start(
                            wqT[:],
                            d[f"wqk_{dom}"][:, T * KT * 128:(T + 1) * KT * 128])
                        pqs = {}
                        for j in jg:
                            pqs[j] = pp.tile([128, 512], F32, tag="pq",
                                             bufs=2, name=f"pq{j}")
                        for kt in range(KT):
                            for j in jg:
                                nc.tensor.matmul(
                                    pqs[j][:], wqT[:, kt * 128:(kt + 1) * 128],
                                    xts[j][:, kt * 512:(kt + 1) * 512],
                                    start=(kt == 0), stop=(kt == KT - 1))
                        for j in jg:
                            pq = pqs[j]
                            # raw bf16 copy (feeds swap matmul + squares)
                            qraw = qp.tile([128, 512], BF16, tag="qraw",
                                           bufs=2)
                            nc.vector.tensor_copy(qraw[:], pq[:])
                            xsq = qp.tile([128, 512], BF16, tag="xsq", bufs=2)
                            nc.vector.tensor_mul(xsq[:], qraw[:], qraw[:])
                            sp = pp.tile([1, 512], F32, tag="sp", bufs=1)
                            nc.tensor.matmul(sp[:], ones_sb[:], xsq[:],
                                             start=True, stop=True)
                            nc.scalar.activation(
                                rms_ins[j][:, (T % 3) * 512:
                                           (T % 3 + 1) * 512],
                                sp[:], AF.Copy)
                            sw = pp.tile([128, 512], F32, tag="sw", bufs=2)
                            nc.tensor.matmul(sw[:], perm_sb[:], qraw[:],
                                             start=True, stop=True)
                            # rope: t12 = raw*cos + swapped*sin   (fp32)
                            cost = qp.tile([128, 512], F32, tag="cs", bufs=2)
                            nc.sync.dma_start(
                                cost[:],
                                d[f"cos_{qk}"][:, j * 512:(j + 1) * 512])
                            sint = qp.tile([128, 512], F32, tag="sn", bufs=2)
                            nc.sync.dma_start(
                                sint[:],
                                d[f"sin_{qk}"][:, j * 512:(j + 1) * 512])
                            t12 = qp.tile([128, 512], F32, tag="t12", bufs=7,
                                          name="t12")
                            t12s[(T, j)] = t12
                            nc.vector.tensor_mul(t12[:], pq[:], cost[:])
                            t2 = qp.tile([128, 512], F32, tag="t2", bufs=1)
                            nc.vector.tensor_mul(t2[:], sw[:], sint[:])
                            nc.vector.tensor_add(t12[:], t12[:], t2[:])
                    # batched r = exp(-0.5*ln(var+eps))
                    for j in jg:
                        lv = qp.tile([1, 3 * 512], F32, tag="lv", bufs=1)
                        nc.scalar.activation(lv[:], rms_ins[j][:], AF.Ln,
                                             scale=1.0 / DH,
                                             bias=eps_sb[:, 0:1])
                        rq = qp.tile([1, 3 * 512], F32, tag="rr", bufs=2)
                        nc.scalar.activation(rq[:], lv[:], AF.Exp, scale=-0.5)
                        for T in Tg:
                            rb = qp.tile([128, 512], F32, tag="rb", bufs=1)
                            nc.gpsimd.partition_broadcast(
                                rb[:], rq[:, (T % 3) * 512:(T % 3 + 1) * 512])
                            nc.vector.tensor_mul(
                                qk_sb[:, T * S + j * 512:
                                      T * S + (j + 1) * 512],
                                t12s[(T, j)][:], rb[:])

                # V in natural layout [seq, dh*3]
                if dom not in wv:
                    wv[dom] = qp.tile([128, KT * IPC], BF16, tag="wv", bufs=1,
                                      name=f"wv_{dom}")
                    nc.sync.dma_start(wv[dom][:], d[f"wv_{dom}"][:])
                for j in jg:
                    for mt in range(j * 4, (j + 1) * 4):
                        pv = pp.tile([128, IPC], F32, tag="pv", bufs=2)
                        for kt in range(KT):
                            nc.tensor.matmul(
                                pv[:],
                                xts[j][:, kt * 512 + (mt % 4) * 128:
                                       kt * 512 + (mt % 4) * 128 + 128],
                                wv[dom][:, kt * IPC:(kt + 1) * IPC],
                                start=(kt == 0), stop=(kt == KT - 1))
                        nc.vector.tensor_copy(
                            v_sb[:, mt * IPC:(mt + 1) * IPC], pv[:])

        # ---------------- phase 2: attention + AllGather + out proj --------
        # chunks: 4x512 + 2x256 (narrow tail to shrink the last-AG bubble)
        CHUNKS = [(0, 512), (512, 512), (1024, 512), (1536, 512),
                  (2048, 256), (2304, 256)]
        with (
            tc.tile_pool(name="att", bufs=1) as ap,
            tc.tile_pool(name="attpsum", bufs=1, space="PSUM") as pp2,
            tc.tile_pool(name="dram", bufs=1, space="DRAM") as dp,
        ):
            wo, bia = {}, {}
            for dom in ("txt", "img"):
                wo[dom] = ap.tile([128, 3 * KT * 128], BF16, tag="wo",
                                  bufs=1, name=f"wo_{dom}")
                nc.sync.dma_start(wo[dom][:], d[f"wo_{dom}"][:])
                bia[dom] = ap.tile([128, 3], F32, tag=f"bias_{dom}", name=f"bia_{dom}"
                                   )
                nc.sync.dma_start(bia[dom][:], d[f"bias_{dom}"][:])

            ag_in = [dp.tile([IPC, W], BF16, tag=f"agi{c}", name=f"agi{c}")
                     for c, (q0, W) in enumerate(CHUNKS)]
            ag_out = [dp.tile([N_CORES * IPC, W], BF16, tag=f"ago{c}",
                              name=f"ago{c}", addr_space="Shared")
                      for c, (q0, W) in enumerate(CHUNKS)]

            def attn_chunk(c):
                q0, W = CHUNKS[c]
                den = ap.tile([1, 3 * 512], F32, tag="den", bufs=1, name="den")
                atu, pts = {}, {}

                def head_qkpv(h):
                    qh = qk_sb[:, h * S + q0:h * S + q0 + W]
                    kh_off = (3 + h) * S
                    pt = ap.tile([128, 20 * 512], BF16, tag="pt", bufs=2,
                                 name="pt")
                    pts[h] = pt
                    for k2 in range(10):
                        sc = pp2.tile([128, 2 * 512], F32, tag="sc", bufs=2,
                                      name="sc")
                        for u in range(2):
                            kt = 2 * k2 + u
                            # W=512: separate banks. W=256: both halves share
                            # bank 0 -- start=True on u=1 would clear u=0's
                            # output, so accumulate-mode overwrite instead.
                            nc.tensor.matmul(
                                sc[:, u * W:(u + 1) * W],
                                qk_sb[:, kh_off + kt * 128:
                                      kh_off + (kt + 1) * 128],
                                qh, start=(W == 512 or u == 0),
                                stop=(W == 512 or u == 1))
                        nc.scalar.activation(
                            pt[:, k2 * 2 * W:(k2 + 1) * 2 * W],
                            sc[:, :2 * W], AF.Exp, scale=SCALE)
                    at = pp2.tile([128, 512], F32, tag="at", bufs=2, name="at")
                    for kt in range(20):
                        last_pv = nc.tensor.matmul(
                            at[:, :W],
                            v_sb[:, kt * IPC + h * 128:
                                 kt * IPC + (h + 1) * 128],
                            pt[:, kt * W:(kt + 1) * W],
                            start=(kt == 0), stop=(kt == 19))
                    attn_chunk.last_pv = last_pv
                    atu[h] = ap.tile([128, 512], F32, tag="atu", bufs=3,
                                     name="atu")
                    nc.vector.tensor_copy(atu[h][:, :W], at[:, :W])

                def head_denom(h):
                    # emitted one head late so the exps + tree-sum of head h
                    # finish while head h+1's QK/PV keeps PE busy
                    pt = pts[h]
                    et = ap.tile([128, 5 * 512], BF16, tag="et", bufs=2,
                                 name="et")
                    nc.vector.tensor_add(et[:, :5 * W], pt[:, :5 * W],
                                         pt[:, 5 * W:10 * W])
                    nc.vector.tensor_add(et[:, :5 * W], et[:, :5 * W],
                                         pt[:, 10 * W:15 * W])
                    nc.vector.tensor_add(et[:, :5 * W], et[:, :5 * W],
                                         pt[:, 15 * W:20 * W])
                    esum = ap.tile([128, 512], BF16, tag="esum", bufs=2,
                                   name="esum")
                    nc.vector.tensor_add(esum[:, :W], et[:, 0:W],
                                         et[:, W:2 * W])
                    for b in range(2, 5):
                        nc.vector.tensor_add(
                            esum[:, :W], esum[:, :W],
                            et[:, b * W:(b + 1) * W])
                    smp = pp2.tile([1, 512], F32, tag="smp", bufs=1,
                                   name="smp")
                    nc.tensor.matmul(smp[:, :W], ones_sb[:], esum[:, :W],
                                     start=True, stop=True)
                    nc.vector.tensor_copy(den[:, h * 512:h * 512 + W],
                                          smp[:, :W])

                for h in range(H_PC):
                    head_qkpv(h)
                    if h >= 1:
                        head_denom(h - 1)
                head_denom(H_PC - 1)

                # batched denominators: one Ln+Exp per chunk
                lden = ap.tile([1, 3 * 512], F32, tag="lden", bufs=1,
                               name="lden")
                nc.scalar.activation(lden[:], den[:], AF.Ln)
                rden = ap.tile([1, 3 * 512], F32, tag="rden", bufs=1,
                               name="rden")
                nc.scalar.activation(rden[:], lden[:], AF.Exp, scale=-1.0)
                for h in range(H_PC):
                    rqb = ap.tile([128, 512], F32, tag="rqb", bufs=2,
                                  name="rqb")
                    nc.gpsimd.partition_broadcast(
                        rqb[:, :W], rden[:, h * 512:h * 512 + W])
                    ast = ap.tile([128, 512], BF16, tag="ast", bufs=2,
                                  name="ast")
                    nc.vector.tensor_mul(ast[:, :W], atu[h][:, :W],
                                         rqb[:, :W])
                    nc.sync.dma_start(
                        ag_in[c][h * 128:(h + 1) * 128, :], ast[:, :W])

            def emit_ag(c):
                nc.gpsimd.collective_compute(
                    "AllGather", mybir.AluOpType.bypass,
                    replica_groups=[list(range(N_CORES))],
                    ins=[ag_in[c].opt()], outs=[ag_out[c].opt()])

            def out_chunk(c, after=None):
                q0, W = CHUNKS[c]
                dom = "txt" if c == 0 else "img"
                gat = ap.tile([128, KT * 512], BF16, tag="gat", bufs=2,
                              name="gat")
                for g in range(4):
                    nc.sync.dma_start(
                        gat[:, g * 6 * W:(g + 1) * 6 * W]
                        .rearrange("p (k q) -> p k q", q=W),
                        ag_out[c][g * 6 * 128:(g + 1) * 6 * 128, :]
                        .rearrange("(k p) q -> p k q", p=128))
                for m in range(3):
                    yo = pp2.tile([128, 512], F32, tag="yo", bufs=1,
                                  name="yo")
                    for kt in range(KT):
                        mm = nc.tensor.matmul(
                            yo[:, :W],
                            wo[dom][:, (m * KT + kt) * 128:
                                    (m * KT + kt + 1) * 128],
                            gat[:, kt * W:(kt + 1) * W],
                            start=(kt == 0), stop=(kt == KT - 1))
                        if after is not None and m == 0 and kt == 0:
                            # keep the out-proj of chunk c behind the
                            # attention matmuls of chunk c+1 on PE, so the
                            # AllGather latency hides behind compute
                            add_dep_helper(mm.ins, after.ins, sync=False,
                                           reason="ag-overlap order")
                    ob = ap.tile([128, 512], F32, tag="ob", bufs=2, name="ob")
                    nc.vector.tensor_scalar_add(ob[:, :W], yo[:, :W],
                                                bia[dom][:, m:m + 1])
                    nc.sync.dma_start(
                        d["yt_out"][m * 128:(m + 1) * 128, q0:q0 + W],
                        ob[:, :W])

            # software-pipelined emission, depth 2: out(c) lands after
            # attn(c+2); AG(c) is triggered after attn(c+1)'s broadcasts so
            # the trigger's DMA wait never blocks them on the gpsimd queue
            NC_ = len(CHUNKS)
            attn_chunk(0)
            emit_ag(0)
            attn_chunk(1)
            emit_ag(1)
            for c in range(2, NC_):
                attn_chunk(c)
                emit_ag(c)
                out_chunk(c - 2, after=attn_chunk.last_pv)
            out_chunk(NC_ - 2, after=None)
            out_chunk(NC_ - 1, after=None)

    nc.compile()
    return nc



def _get_nc():
    if "nc" not in _CACHE:
        _CACHE["nc"] = _build()
    return _CACHE["nc"]


# ---------------------------------------------------------------- entry points

def kernel_run(inputs, trace=False):
    inp = {k: np.asarray(v) for k, v in inputs.items()}
    in_maps = _prep_inputs(inp)
    nc = _get_nc()
    res = run_bass_kernel_spmd(nc, in_maps, core_ids=list(range(N_CORES)),
                               trace=trace)
    yt = np.concatenate([res.results[c]["yt_out"] for c in range(N_CORES)],
                        axis=0)                     # [3072, 2560]
    y = np.ascontiguousarray(yt.T, dtype=np.float32)  # [2560, 3072]
    img_out = y[None, S_TXT:, :]
    txt_out = y[None, :S_TXT, :]
    return (np.ascontiguousarray(img_out), np.ascontiguousarray(txt_out)), res


def kernel(**inputs):
    out, _ = kernel_run(inputs, trace=False)
    return out


# revision 47
# speedup vs baseline: 1.0666x; 1.0666x over previous
"""FLUX-style joint attention block (SVDQuant linears) on 8 TRN2 NeuronCores.

Strategy (tensor parallel over heads, per the sharding hint):
  - 24 heads -> 3 heads per core. Each core computes its QKV column shard
    from the full (replicated) hidden states, runs attention for its heads,
    then computes its 384 output-feature rows of the output projections in a
    transposed formulation. Head outputs are exchanged with a per-chunk
    AllGather of attn^T (cheaper than all-reducing the full output, and
    overlapped with attention compute of the next chunk).
  - All GEMMs in bf16 with fp32 PSUM accumulation. RMS-norm + RoPE applied
    in fp32 on the transposed [head_dim, seq] layout: partition-axis sums
    via ones-matmul, RoPE pair swap via a permutation matmul, per-column
    scale broadcast via gpsimd.partition_broadcast.
  - softmax without max-subtraction (scores are O(5) here); denominators
    from bf16 tree-sum + ones-matmul; normalization folded into the PSUM
    eviction of P@V.

Self-contained: hardcodes shapes from the problem spec.
"""

import os
import sys
from contextlib import ExitStack

import numpy as np
import ml_dtypes

for _p in ("/opt/trn_rl_repo",):
    if os.path.isdir(_p) and _p not in sys.path:
        sys.path.append(_p)

import concourse.bass as bass  # noqa: E402
import concourse.mybir as mybir  # noqa: E402
import concourse.tile as tile  # noqa: E402
from concourse.tile import add_dep_helper  # noqa: E402
from concourse import bacc  # noqa: E402
from concourse import tile_utils  # noqa: E402
from concourse.bass_utils import run_bass_kernel_spmd  # noqa: E402

# Use the full usable SBUF (224 KiB phys / 208 usable per partition).
tile_utils.max_sbuf_usage = 208 * 1024

N_CORES = 8
S_TXT, S_IMG = 512, 2048
S = S_TXT + S_IMG                      # 2560
QD = 3072                              # model dim
HEADS, DH, RANK = 24, 128, 32
H_PC = HEADS // N_CORES                # 3 heads per core
IPC = H_PC * DH                        # 384 inner dims per core
KT = QD // 128                         # 24 contraction tiles
NT = S // 512                          # 5 seq tiles of 512 (tile 0 = txt)
HALVES = [(0, 3), (3, 2)]              # (first n-tile, n n-tiles): 1536 + 1024
SCALE = 1.0 / float(np.sqrt(np.float32(DH)))
EPS = 1e-5

BF16 = mybir.dt.bfloat16
F32 = mybir.dt.float32
AF = mybir.ActivationFunctionType

_CACHE = {}


# ---------------------------------------------------------------- host packing

def _bf(a):
    return np.ascontiguousarray(a.astype(ml_dtypes.bfloat16))


def _f32(a):
    return np.ascontiguousarray(a.astype(np.float32))


def _pack_st(a):
    """[3072, M] -> [128, KT*M] with block kt at columns [kt*M, (kt+1)*M)."""
    m = a.shape[1]
    return a.reshape(KT, 128, m).transpose(1, 0, 2).reshape(128, KT * m)


def _prep_inputs(inp):
    """Build the 8 per-core input maps (all np arrays, bf16/f32)."""
    x = np.concatenate([inp["encoder_hidden_states"][0],
                        inp["hidden_states"][0]], axis=0).astype(np.float32)

    # xT packed n-tile-major: block j is [128, KT*512], kt-blocks inside.
    xt_blocks = []
    for j in range(NT):
        xs = x[j * 512:(j + 1) * 512]             # [512, 3072]
        xt_blocks.append(xs.T.reshape(KT, 128, 512)
                         .transpose(1, 0, 2).reshape(128, KT * 512))
    xt_pack = _bf(np.concatenate(xt_blocks, axis=1))

    rope = np.asarray(inp["image_rotary_emb"], np.float32)   # [S, 64, 2]
    cos = np.repeat(rope[:, :, 0], 2, axis=1)                # [S, 128]
    sinE = np.empty((S, DH), np.float32)
    sinE[:, 0::2] = -rope[:, :, 1]
    sinE[:, 1::2] = rope[:, :, 1]
    txt_rows = (np.arange(S) < S_TXT)[:, None]
    gq = np.where(txt_rows, inp["g_aq"][None, :], inp["g_q"][None, :])
    gk = np.where(txt_rows, inp["g_ak"][None, :], inp["g_k"][None, :])
    swap = lambda a: a[:, np.arange(DH) ^ 1]
    cs = {
        "cos_q": (cos * gq).T, "sin_q": (sinE * swap(gq)).T,
        "cos_k": (cos * gk).T, "sin_k": (sinE * swap(gk)).T,
    }  # each [128, S] fp32

    perm = np.zeros((128, 128), np.float32)
    perm[np.arange(128), np.arange(128) ^ 1] = 1.0
    ones = np.ones((128, 1), np.float32)

    # fold the SVD low-rank branch into the weights: w_eff = w + pd @ pu
    w_qkv = inp["w_qkv"] + inp["pd_qkv"] @ inp["pu_qkv"]
    w_aqkv = inp["w_add_qkv"] + inp["pd_add_qkv"] @ inp["pu_add_qkv"]
    w_out = inp["w_out"] + inp["pd_out"] @ inp["pu_out"]
    w_aout = inp["w_add_out"] + inp["pd_add_out"] @ inp["pu_add_out"]

    in_maps = []
    for c in range(N_CORES):
        heads = [3 * c + j for j in range(H_PC)]
        qk_cols = np.concatenate(
            [np.arange(h * 128, (h + 1) * 128) for h in heads]
            + [np.arange(QD + h * 128, QD + (h + 1) * 128) for h in heads])
        v_cols = np.concatenate(
            [np.arange(2 * QD + h * 128, 2 * QD + (h + 1) * 128)
             for h in heads])
        qd_cols = np.arange(c * IPC, (c + 1) * IPC)

        def pack_mtiles(w, cols, mt):
            sel = w[:, cols]
            return np.concatenate(
                [_pack_st(sel[:, m * 128:(m + 1) * 128]) for m in range(mt)],
                axis=1)

        m = {
            "xt": xt_pack,
            "wqk_img": _bf(pack_mtiles(w_qkv, qk_cols, 6)),
            "wqk_txt": _bf(pack_mtiles(w_aqkv, qk_cols, 6)),
            "wv_img": _bf(_pack_st(w_qkv[:, v_cols])),
            "wv_txt": _bf(_pack_st(w_aqkv[:, v_cols])),
            "wo_img": _bf(pack_mtiles(w_out, qd_cols, 3)),
            "wo_txt": _bf(pack_mtiles(w_aout, qd_cols, 3)),
            "bias_img": _f32(inp["b_out"][qd_cols].reshape(3, 128).T),
            "bias_txt": _f32(inp["b_add_out"][qd_cols].reshape(3, 128).T),
            "cos_q": _f32(cs["cos_q"]), "sin_q": _f32(cs["sin_q"]),
            "cos_k": _f32(cs["cos_k"]), "sin_k": _f32(cs["sin_k"]),
            "permT": _bf(perm), "ones128": _bf(ones),
        }
        in_maps.append(m)
    return in_maps


# ---------------------------------------------------------------- device build

def _declare(nc):
    d = {}
    def inp(name, shape, dt=BF16):
        d[name] = nc.dram_tensor(name, list(shape), dt, kind="ExternalInput")
    inp("xt", [128, NT * KT * 512])
    inp("wqk_img", [128, 6 * KT * 128]); inp("wqk_txt", [128, 6 * KT * 128])
    inp("wv_img", [128, KT * IPC]); inp("wv_txt", [128, KT * IPC])
    inp("wo_img", [128, 3 * KT * 128]); inp("wo_txt", [128, 3 * KT * 128])
    inp("bias_img", [128, 3], F32); inp("bias_txt", [128, 3], F32)
    for n in ("cos_q", "sin_q", "cos_k", "sin_k"):
        inp(n, [128, S], F32)
    inp("permT", [128, 128]); inp("ones128", [128, 1])
    d["yt_out"] = nc.dram_tensor("yt_out", [IPC, S], F32,
                                 kind="ExternalOutput")
    return d


def _build():
    nc = bacc.Bacc("TRN2", target_bir_lowering=False, debug=False,
                   num_devices=N_CORES)
    d = _declare(nc)

    with tile.TileContext(nc) as tc, ExitStack() as ctx:
        persist = ctx.enter_context(tc.tile_pool(name="persist", bufs=1))
        # persistent tensors
        qk_sb = persist.tile([128, 6 * S], BF16, tag="qk")   # [T][d, seq]
        v_sb = persist.tile([128, NT * 4 * IPC], BF16, tag="v")  # 20 seq tiles
        ones_sb = persist.tile([128, 1], BF16, tag="ones")
        perm_sb = persist.tile([128, 128], BF16, tag="perm")
        eps_sb = persist.tile([1, 1], F32, tag="eps")
        nc.sync.dma_start(ones_sb[:], d["ones128"][:])
        nc.sync.dma_start(perm_sb[:], d["permT"][:])
        nc.vector.memset(eps_sb[:], EPS)

        # ---------------- phase 1: QKV + rms/rope, n-tile-outer ------------
        with (
            tc.tile_pool(name="qkvtmp", bufs=1) as qp,
            tc.tile_pool(name="qkvpsum", bufs=1, space="PSUM") as pp,
        ):
            wv = {}

            for jg in ((0,), (1, 2), (3, 4)):
                dom = "txt" if jg[0] == 0 else "img"
                xts = {}
                first_group = (jg == (0,))
                for j in jg:
                    xts[j] = qp.tile([128, KT * 512], BF16, tag="xt",
                                     bufs=2, name=f"xt{j}")
                    # split so the first k-tiles land early and matmuls start
                    cut = 4 * 512
                    nc.sync.dma_start(
                        xts[j][:, :cut],
                        d["xt"][:, j * KT * 512:j * KT * 512 + cut])
                    nc.sync.dma_start(
                        xts[j][:, cut:],
                        d["xt"][:, j * KT * 512 + cut:(j + 1) * KT * 512])

                # Q,K transposed GEMM; one LDWEIGHTS serves the j-pair.
                # rms ln/exp batched over T-groups of 3 to avoid ACT table
                # set thrashing between Ln and Exp.
                for Tg in ((0, 1, 2), (3, 4, 5)):
                    rms_ins, t12s = {}, {}
                    for j in jg:
                        rms_ins[j] = qp.tile([1, 3 * 512], F32, tag="rms_in",
                                             bufs=2, name=f"rms_in{j}")
                    for T in Tg:
                        qk = "q" if T < 3 else "k"
                        wqT = qp.tile([128, KT * 128], BF16, tag="wq",
                                      bufs=3, name="wqT")
                        nc.sync.dma_start(
                            wqT[:],
                            d[f"wqk_{dom}"][:, T * KT * 128:(T + 1) * KT * 128])
                        pqs = {}
                        for j in jg:
                            pqs[j] = pp.tile([128, 512], F32, tag="pq",
                                             bufs=2, name=f"pq{j}")
                        for kt in range(KT):
                            for j in jg:
                                nc.tensor.matmul(
                                    pqs[j][:], wqT[:, kt * 128:(kt + 1) * 128],
                                    xts[j][:, kt * 512:(kt + 1) * 512],
                                    start=(kt == 0), stop=(kt == KT - 1))
                        for j in jg:
                            pq = pqs[j]
                            # raw bf16 copy (feeds swap matmul + squares)
                            qraw = qp.tile([128, 512], BF16, tag="qraw",
                                           bufs=2)
                            nc.vector.tensor_copy(qraw[:], pq[:])
                            xsq = qp.tile([128, 512], BF16, tag="xsq", bufs=2)
                            nc.vector.tensor_mul(xsq[:], qraw[:], qraw[:])
                            sp = pp.tile([1, 512], F32, tag="sp", bufs=1)
                            nc.tensor.matmul(sp[:], ones_sb[:], xsq[:],
                                             start=True, stop=True)
                            nc.scalar.activation(
                                rms_ins[j][:, (T % 3) * 512:
                                           (T % 3 + 1) * 512],
                                sp[:], AF.Copy)
                            sw = pp.tile([128, 512], F32, tag="sw", bufs=2)
                            nc.tensor.matmul(sw[:], perm_sb[:], qraw[:],
                                             start=True, stop=True)
                            # rope: t12 = raw*cos + swapped*sin   (fp32)
                            cost = qp.tile([128, 512], F32, tag="cs", bufs=2)
                            nc.sync.dma_start(
                                cost[:],
                                d[f"cos_{qk}"][:, j * 512:(j + 1) * 512])
                            sint = qp.tile([128, 512], F32, tag="sn", bufs=2)
                            nc.sync.dma_start(
                                sint[:],
                                d[f"sin_{qk}"][:, j * 512:(j + 1) * 512])
                            t12 = qp.tile([128, 512], F32, tag="t12", bufs=7,
                                          name="t12")
                            t12s[(T, j)] = t12
                            nc.vector.tensor_mul(t12[:], pq[:], cost[:])
                            t2 = qp.tile([128, 512], F32, tag="t2", bufs=1)
                            nc.vector.tensor_mul(t2[:], sw[:], sint[:])
                            nc.vector.tensor_add(t12[:], t12[:], t2[:])
                    # batched r = exp(-0.5*ln(var+eps))
                    for j in jg:
                        lv = qp.tile([1, 3 * 512], F32, tag="lv", bufs=1)
                        nc.scalar.activation(lv[:], rms_ins[j][:], AF.Ln,
                                             scale=1.0 / DH,
                                             bias=eps_sb[:, 0:1])
                        rq = qp.tile([1, 3 * 512], F32, tag="rr", bufs=2)
                        nc.scalar.activation(rq[:], lv[:], AF.Exp, scale=-0.5)
                        for T in Tg:
                            rb = qp.tile([128, 512], F32, tag="rb", bufs=1)
                            nc.gpsimd.partition_broadcast(
                                rb[:], rq[:, (T % 3) * 512:(T % 3 + 1) * 512])
                            nc.vector.tensor_mul(
                                qk_sb[:, T * S + j * 512:
                                      T * S + (j + 1) * 512],
                                t12s[(T, j)][:], rb[:])

                # V in natural layout [seq, dh*3]
                if dom not in wv:
                    wv[dom] = qp.tile([128, KT * IPC], BF16, tag="wv", bufs=2,
                                      name=f"wv_{dom}")
                    nc.sync.dma_start(wv[dom][:], d[f"wv_{dom}"][:])
                for j in jg:
                    for mt in range(j * 4, (j + 1) * 4):
                        pv = pp.tile([128, IPC], F32, tag="pv", bufs=2)
                        for kt in range(KT):
                            nc.tensor.matmul(
                                pv[:],
                                xts[j][:, kt * 512 + (mt % 4) * 128:
                                       kt * 512 + (mt % 4) * 128 + 128],
                                wv[dom][:, kt * IPC:(kt + 1) * IPC],
                                start=(kt == 0), stop=(kt == KT - 1))
                        nc.vector.tensor_copy(
                            v_sb[:, mt * IPC:(mt + 1) * IPC], pv[:])

        # ---------------- phase 2: attention + AllGather + out proj --------
        # chunks: 4x512 + 2x256 (narrow tail to shrink the last-AG bubble)
        CHUNKS = [(0, 512), (512, 512), (1024, 512), (1536, 512),
                  (2048, 256), (2304, 256)]
        with (
            tc.tile_pool(name="att", bufs=1) as ap,
            tc.tile_pool(name="attpsum", bufs=1, space="PSUM") as pp2,
            tc.tile_pool(name="dram", bufs=1, space="DRAM") as dp,
        ):
            wo, bia = {}, {}
            for dom in ("txt", "img"):
                wo[dom] = ap.tile([128, 3 * KT * 128], BF16, tag="wo",
                                  bufs=1, name=f"wo_{dom}")
                nc.sync.dma_start(wo[dom][:], d[f"wo_{dom}"][:])
                bia[dom] = ap.tile([128, 3], F32, tag=f"bias_{dom}", name=f"bia_{dom}"
                                   )
                nc.sync.dma_start(bia[dom][:], d[f"bias_{dom}"][:])

            ag_in = [dp.tile([IPC, W], BF16, tag=f"agi{c}", name=f"agi{c}")
                     for c, (q0, W) in enumerate(CHUNKS)]
            ag_out = [dp.tile([N_CORES * IPC, W], BF16, tag=f"ago{c}",
                              name=f"ago{c}", addr_space="Shared")
                      for c, (q0, W) in enumerate(CHUNKS)]

            def attn_chunk(c):
                q0, W = CHUNKS[c]
                den = ap.tile([1, 3 * 512], F32, tag="den", bufs=1, name="den")
                atu, pts = {}, {}

                def head_qkpv(h):
                    qh = qk_sb[:, h * S + q0:h * S + q0 + W]
                    kh_off = (3 + h) * S
                    pt = ap.tile([128, 20 * 512], BF16, tag="pt", bufs=2,
                                 name="pt")
                    pts[h] = pt
                    for k2 in range(10):
                        sc = pp2.tile([128, 2 * 512], F32, tag="sc", bufs=2,
                                      name="sc")
                        for u in range(2):
                            kt = 2 * k2 + u
                            # W=512: separate banks. W=256: both halves share
                            # bank 0 -- start=True on u=1 would clear u=0's
                            # output, so accumulate-mode overwrite instead.
                            nc.tensor.matmul(
                                sc[:, u * W:(u + 1) * W],
                                qk_sb[:, kh_off + kt * 128:
                                      kh_off + (kt + 1) * 128],
                                qh, start=(W == 512 or u == 0),
                                stop=(W == 512 or u == 1))
                        nc.scalar.activation(
                            pt[:, k2 * 2 * W:(k2 + 1) * 2 * W],
                            sc[:, :2 * W], AF.Exp, scale=SCALE)
                    at = pp2.tile([128, 512], F32, tag="at", bufs=2, name="at")
                    for kt in range(20):
                        last_pv = nc.tensor.matmul(
                            at[:, :W],
                            v_sb[:, kt * IPC + h * 128:
                                 kt * IPC + (h + 1) * 128],
                            pt[:, kt * W:(kt + 1) * W],
                            start=(kt == 0), stop=(kt == 19))
                    attn_chunk.last_pv = last_pv
                    atu[h] = ap.tile([128, 512], F32, tag="atu", bufs=3,
                                     name="atu")
                    nc.vector.tensor_copy(atu[h][:, :W], at[:, :W])

                def head_denom(h):
                    # emitted one head late so the exps + tree-sum of head h
                    # finish while head h+1's QK/PV keeps PE busy
                    pt = pts[h]
                    et = ap.tile([128, 5 * 512], BF16, tag="et", bufs=2,
                                 name="et")
                    nc.vector.tensor_add(et[:, :5 * W], pt[:, :5 * W],
                                         pt[:, 5 * W:10 * W])
                    nc.vector.tensor_add(et[:, :5 * W], et[:, :5 * W],
                                         pt[:, 10 * W:15 * W])
                    nc.vector.tensor_add(et[:, :5 * W], et[:, :5 * W],
                                         pt[:, 15 * W:20 * W])
                    esum = ap.tile([128, 512], BF16, tag="esum", bufs=2,
                                   name="esum")
                    nc.vector.tensor_add(esum[:, :W], et[:, 0:W],
                                         et[:, W:2 * W])
                    for b in range(2, 5):
                        nc.vector.tensor_add(
                            esum[:, :W], esum[:, :W],
                            et[:, b * W:(b + 1) * W])
                    smp = pp2.tile([1, 512], F32, tag="smp", bufs=1,
                                   name="smp")
                    nc.tensor.matmul(smp[:, :W], ones_sb[:], esum[:, :W],
                                     start=True, stop=True)
                    nc.vector.tensor_copy(den[:, h * 512:h * 512 + W],
                                          smp[:, :W])

                for h in range(H_PC):
                    head_qkpv(h)
                    if h >= 1:
                        head_denom(h - 1)
                head_denom(H_PC - 1)

                # batched denominators: one Ln+Exp per chunk
                lden = ap.tile([1, 3 * 512], F32, tag="lden", bufs=1,
                               name="lden")
                nc.scalar.activation(lden[:], den[:], AF.Ln)
                rden = ap.tile([1, 3 * 512], F32, tag="rden", bufs=1,
                               name="rden")
                nc.scalar.activation(rden[:], lden[:], AF.Exp, scale=-1.0)
                for h in range(H_PC):
                    rqb = ap.tile([128, 512], F32, tag="rqb", bufs=2,
                                  name="rqb")
                    nc.gpsimd.partition_broadcast(
                        rqb[:, :W], rden[:, h * 512:h * 512 + W])
                    ast = ap.tile([128, 512], BF16, tag="ast", bufs=2,
                                  name="ast")
                    nc.vector.tensor_mul(ast[:, :W], atu[h][:, :W],
                                         rqb[:, :W])
                    nc.sync.dma_start(
                        ag_in[c][h * 128:(h + 1) * 128, :], ast[:, :W])

            def emit_ag(c):
                nc.gpsimd.collective_compute(
                    "AllGather", mybir.AluOpType.bypass,
                    replica_groups=[list(range(N_CORES))],
                    ins=[ag_in[c].opt()], outs=[ag_out[c].opt()])

            def out_chunk(c, after=None):
                q0, W = CHUNKS[c]
                dom = "txt" if c == 0 else "img"
                gat = ap.tile([128, KT * 512], BF16, tag="gat", bufs=2,
                              name="gat")
                for g in range(4):
                    nc.sync.dma_start(
                        gat[:, g * 6 * W:(g + 1) * 6 * W]
                        .rearrange("p (k q) -> p k q", q=W),
                        ag_out[c][g * 6 * 128:(g + 1) * 6 * 128, :]
                        .rearrange("(k p) q -> p k q", p=128))
                for m in range(3):
                    yo = pp2.tile([128, 512], F32, tag="yo", bufs=1,
                                  name="yo")
                    for kt in range(KT):
                        mm = nc.tensor.matmul(
                            yo[:, :W],
                            wo[dom][:, (m * KT + kt) * 128:
                                    (m * KT + kt + 1) * 128],
                            gat[:, kt * W:(kt + 1) * W],
                            start=(kt == 0), stop=(kt == KT - 1))
                        if after is not None and m == 0 and kt == 0:
                            # keep the out-proj of chunk c behind the
                            # attention matmuls of chunk c+1 on PE, so the
                            # AllGather latency hides behind compute
                            add_dep_helper(mm.ins, after.ins, sync=False,
                                           reason="ag-overlap order")
                    ob = ap.tile([128, 512], F32, tag="ob", bufs=2, name="ob")
                    nc.vector.tensor_scalar_add(ob[:, :W], yo[:, :W],
                                                bia[dom][:, m:m + 1])
                    nc.sync.dma_start(
                        d["yt_out"][m * 128:(m + 1) * 128, q0:q0 + W],
                        ob[:, :W])

            # software-pipelined emission, depth 2: out(c) lands after
            # attn(c+2); AG(c) is triggered after attn(c+1)'s broadcasts so
            # the trigger's DMA wait never blocks them on the gpsimd queue
            NC_ = len(CHUNKS)
            attn_chunk(0)
            emit_ag(0)
            attn_chunk(1)
            emit_ag(1)
            for c in range(2, NC_):
                attn_chunk(c)
                emit_ag(c)
                out_chunk(c - 2, after=attn_chunk.last_pv)
            out_chunk(NC_ - 2, after=None)
            out_chunk(NC_ - 1, after=None)

    nc.compile()
    return nc



def _get_nc():
    if "nc" not in _CACHE:
        _CACHE["nc"] = _build()
    return _CACHE["nc"]


# ---------------------------------------------------------------- entry points

def kernel_run(inputs, trace=False):
    inp = {k: np.asarray(v) for k, v in inputs.items()}
    in_maps = _prep_inputs(inp)
    nc = _get_nc()
    res = run_bass_kernel_spmd(nc, in_maps, core_ids=list(range(N_CORES)),
                               trace=trace)
    yt = np.concatenate([res.results[c]["yt_out"] for c in range(N_CORES)],
                        axis=0)                     # [3072, 2560]
    y = np.ascontiguousarray(yt.T, dtype=np.float32)  # [2560, 3072]
    img_out = y[None, S_TXT:, :]
    txt_out = y[None, :S_TXT, :]
    return (np.ascontiguousarray(img_out), np.ascontiguousarray(txt_out)), res


def kernel(**inputs):
    out, _ = kernel_run(inputs, trace=False)
    return out
